# revision 26
# baseline (speedup 1.0000x reference)
"""Bayesian linear layer (reparameterized sample + KL) on 8 Trainium2 NeuronCores.

Reference computation (all fp32):
    weight = weight_mu + exp(weight_sigma) * eps_w          # [OUT, IN]
    bias   = bias_mu   + exp(bias_sigma)   * eps_b          # [OUT]
    out    = x @ weight.T + bias                            # [B, OUT]
    kl     = -0.5 * sum(1 + 2*ws - wm^2 - exp(2*ws))        # over weight
             -0.5 * sum(1 + 2*bs - bm^2 - exp(2*bs))        # over bias

Sharding: column-parallel 1x8.  out_features split in 8 (O_S=512 per core), x
replicated.  Per core: W^T shard [4096, 512] stays resident in SBUF (64KB per
partition, f32r), x streams through in 64 batch tiles.  The weight-param
stream is only 25MB/core so the W-generation phase is short, and 4 batch
tiles run a PE "prologue" during it (k-sliced PSUM accumulation across all 8
banks) so the tensor engine never goes cold.

Host-side layout work (part of the sharding strategy):
  - x is passed pre-transposed AND pre-tiled as [64, 128, 32, 128]
    ([batch-tile, feature-in-ktile, ktile, batch-in-tile]) so each batch-tile
    load is one DMA with 16KB-contiguous per-partition runs.
  - weight mu/sigma/eps shards are interleaved per k-tile as [32, 128, 3, 512]
    (6KB-contiguous per partition) -> one DMA per k-tile.
The contraction dim lands on SBUF partitions with no on-device transposes.

Matmuls run in float32r (the PE's full-rate fp32 mode, tf32-like operand
rounding, fp32 PSUM accumulation).
"""

import sys

import numpy as np

try:
    import concourse.bass as bass  # noqa: F401
except ImportError:  # pragma: no cover
    sys.path.insert(0, "/opt/trn_rl_repo")

import concourse.bass as bass
import concourse.tile as tile
from concourse import bacc, bass_isa, mybir

P = 128
B_FULL, IN_FULL, OUT_FULL = 8192, 4096, 4096
O_SHARDS = 8
N_CORES = 8

F32 = mybir.dt.float32
# fp16 operands: 11-bit significand, the same precision class as the PE's
# tf32-like f32r operand rounding, at half the DMA bytes and SBUF footprint.
X_DT = mybir.dt.float16
W_DT = mybir.dt.float16
MM_DT = X_DT

AF = mybir.ActivationFunctionType
OP = mybir.AluOpType
AX = mybir.AxisListType


def build_bayes_kernel(nc, IN, B_S, O_S, mm_dt=X_DT, w_dt=W_DT):
    """Emit the per-core SPMD program. Tensors are declared on nc."""
    KT = IN // P        # k tiles
    BT = B_S // P       # batch tiles
    assert O_S <= 512   # one psum bank per out tile

    xt = nc.dram_tensor("xt", [BT, P, KT, P], mm_dt, kind="ExternalInput").ap()
    wpk = nc.dram_tensor("wpk", [KT, P, 3, O_S], F32, kind="ExternalInput").ap()
    bpk = nc.dram_tensor("bpk", [1, 3, O_S], F32, kind="ExternalInput").ap()
    out = nc.dram_tensor("out", [B_S, O_S], F32, kind="ExternalOutput").ap()
    kl = nc.dram_tensor("kl", [1, 1], F32, kind="ExternalOutput").ap()

    out_r = out.rearrange("(bt p) o -> p bt o", p=P)

    with tile.TileContext(nc) as tc:
        with (
            tc.tile_pool(name="wpool", bufs=1) as wpool,
            tc.tile_pool(name="gen", bufs=4) as gen,
            tc.tile_pool(name="xin", bufs=10) as xin,
            tc.tile_pool(name="ost", bufs=3) as ost,
            tc.tile_pool(name="misc", bufs=1) as misc,
            tc.tile_pool(name="psum", bufs=8, space="PSUM") as psum,
        ):
            # Persistent state
            w_tiles = [
                wpool.tile([P, O_S], w_dt, tag=f"w{kt}", name=f"w{kt}")
                for kt in range(KT)
            ]
            ssig = misc.tile([P, KT], F32, tag="ssig")   # per-ktile sum(sigma)
            smu2 = misc.tile([P, KT], F32, tag="smu2")   # sum(mu^2)
            sv2 = misc.tile([P, KT], F32, tag="sv2")     # sum(exp(2 sigma))
            b_bc = misc.tile([P, O_S], F32, tag="bbc")   # bias broadcast
            klb = misc.tile([1, 4], F32, tag="klb")      # bias kl: ssig, smu2, sv2, tmp

            # Prefetch x tiles on the sync ring; they feed the PE prologue
            # that runs inside phase 1.
            PG = min(8, BT)
            xpre = []
            for bt in range(PG):
                xs = xin.tile([P, KT, P], mm_dt, tag="xs", name=f"xpre{bt}")
                nc.sync.dma_start(xs, xt[bt])
                xpre.append(xs)
            pg_ps = [
                psum.tile([P, O_S], F32, tag="ps", name=f"pg{pb}")
                for pb in range(PG)
            ]

            # ---- Phase 1: W = mu + exp(sigma)*eps, KL partial sums ----
            # ACT ops depend only on the k-tile's DMA (dumps go to dedicated
            # scratch), DVE work runs back-to-back, and the 4-deep gen pool
            # pipelines the DMA->ACT->DVE chain across k-tiles.  The first PG
            # batch tiles accumulate their matmuls k-tile by k-tile here so
            # the PE consumes each W tile as it is produced.
            for kt in range(KT):
                g = gen.tile([P, 3, O_S], F32, tag="wpk")
                nc.sync.dma_start(g, wpk[kt])
                sig, mu, eps = g[:, 0, :], g[:, 1, :], g[:, 2, :]
                wtmp = gen.tile([P, O_S], F32, tag="wtmp")
                nc.scalar.activation(wtmp, sig, AF.Exp)             # v
                d2 = misc.tile([P, O_S], F32, tag="sqd")
                nc.scalar.activation(
                    d2, mu, AF.Square, accum_out=smu2[:, kt : kt + 1]
                )
                nc.vector.tensor_reduce(ssig[:, kt : kt + 1], sig, AX.X, OP.add)
                # v^2 summed; tensor output dumps over the dead sigma slot
                nc.vector.scalar_tensor_tensor(
                    sig, wtmp, 1.0, wtmp, OP.mult, OP.mult,
                    accum_out=sv2[:, kt : kt + 1],
                )
                nc.vector.tensor_tensor(wtmp, wtmp, eps, OP.mult)
                # final add writes the f32r matmul operand (single rounding)
                nc.vector.tensor_tensor(w_tiles[kt], wtmp, mu, OP.add)
                for pb in range(PG):
                    nc.tensor.matmul(
                        pg_ps[pb],
                        xpre[pb][:, kt, :],
                        w_tiles[kt],
                        start=(kt == 0),
                        stop=(kt == KT - 1),
                    )

            # ---- Bias: value + KL pieces ----
            # all SBUF operands of an op share start partition 0; dead slots
            # of the packed tile double as ACT dump targets.
            bt_ = gen.tile([1, 3, O_S], F32, tag="wpk", name="biastile")
            nc.sync.dma_start(bt_, bpk[0])
            bsig, bmu, beps = bt_[:, 0, :], bt_[:, 1, :], bt_[:, 2, :]
            bv = b_bc[0:1, :]
            nc.vector.tensor_reduce(klb[:, 0:1], bsig, AX.X, OP.add)
            nc.scalar.activation(bv, bsig, AF.Exp)
            nc.vector.tensor_tensor(bv, bv, beps, OP.mult)   # beps dead after
            nc.vector.tensor_tensor(bv, bv, bmu, OP.add)
            nc.scalar.activation(
                beps, bsig, AF.Exp, scale=2.0, accum_out=klb[:, 2:3]
            )
            nc.scalar.activation(bsig, bmu, AF.Square, accum_out=klb[:, 1:2])
            nc.gpsimd.partition_broadcast(b_bc, bv)

            # ---- Prologue eviction: bias-add + store for the PG tiles ----
            for pb in range(PG):
                osb = ost.tile([P, O_S], F32, tag="osb", name=f"osb_pg{pb}")
                nc.vector.tensor_tensor(osb, pg_ps[pb], b_bc, OP.add)
                nc.sync.dma_start(out_r[:, pb, :], osb)

            # ---- Phase 2: out[bt] = x[bt] @ W^T + bias ----
            for bt in range(PG, BT):
                xs = xin.tile([P, KT, P], mm_dt, tag="xs")
                nc.sync.dma_start(xs, xt[bt])
                ps = psum.tile([P, O_S], F32, tag="ps")
                for kt in range(KT):
                    nc.tensor.matmul(
                        ps,
                        xs[:, kt, :],
                        w_tiles[kt],
                        start=(kt == 0),
                        stop=(kt == KT - 1),
                    )
                osb = ost.tile([P, O_S], F32, tag="osb")
                nc.vector.tensor_tensor(osb, ps, b_bc, OP.add)
                nc.sync.dma_start(out_r[:, bt, :], osb)

            # ---- KL tail ----
            rs = misc.tile([P, 1], F32, tag="rs")
            rm = misc.tile([P, 1], F32, tag="rm")
            rv = misc.tile([P, 1], F32, tag="rv")
            nc.vector.tensor_reduce(rs, ssig, AX.X, OP.add)
            nc.vector.tensor_reduce(rm, smu2, AX.X, OP.add)
            nc.vector.tensor_reduce(rv, sv2, AX.X, OP.add)
            tcol = misc.tile([P, 1], F32, tag="tcol")
            # tcol = 2*rs - rm - rv
            nc.vector.scalar_tensor_tensor(tcol, rs, 2.0, rm, OP.mult, OP.subtract)
            nc.vector.tensor_tensor(tcol, tcol, rv, OP.subtract)
            # bias terms fold into partition 0
            nc.vector.scalar_tensor_tensor(
                klb[:, 3:4], klb[:, 0:1], 2.0, klb[:, 1:2], OP.mult, OP.subtract
            )
            nc.vector.tensor_tensor(klb[:, 3:4], klb[:, 3:4], klb[:, 2:3], OP.subtract)
            nc.vector.tensor_tensor(tcol[0:1, :], tcol[0:1, :], klb[:, 3:4], OP.add)
            tall = misc.tile([P, 1], F32, tag="tall")
            nc.gpsimd.partition_all_reduce(tall, tcol, P, bass_isa.ReduceOp.add)
            # kl = -0.5 * (count + sum(2s - m^2 - v^2))
            count = float(IN * O_S + O_S)
            klt = misc.tile([1, 1], F32, tag="klt")
            nc.vector.tensor_scalar(klt, tall[0:1, :], count, -0.5, OP.add, OP.mult)
            nc.sync.dma_start(kl, klt)


_NC_CACHE = {}


def _get_nc():
    key = "full"
    if key not in _NC_CACHE:
        nc = bacc.Bacc("TRN2", target_bir_lowering=False, debug=False)
        build_bayes_kernel(nc, IN_FULL, B_FULL, OUT_FULL // O_SHARDS)
        nc.compile()
        _NC_CACHE[key] = nc
    return _NC_CACHE[key]


def _pack_x(x_full, BT, KT):
    # [B, IN] -> [BT, P, KT, P] with [bt, p(=feature in tile), kt, b], fp16
    x4 = x_full.reshape(BT, P, KT, P)           # [bt, b, kt, p_feature]
    return np.ascontiguousarray(x4.transpose(0, 3, 2, 1).astype(np.float16))


def _pack_w(sig, mu, eps, KT, O_S):
    # each [O_S, IN] -> packed [KT, P, 3, O_S] with feature on partition
    stk = np.stack([sig.T, mu.T, eps.T], axis=1)   # [IN, 3, O_S]
    return np.ascontiguousarray(stk.reshape(KT, P, 3, O_S))


def _shard_inputs(x, weight_mu, weight_sigma, bias_mu, bias_sigma, eps_w, eps_b):
    O_S = OUT_FULL // O_SHARDS
    BT, KT = B_FULL // P, IN_FULL // P
    f = np.float32
    x = np.asarray(x, dtype=f)
    weight_mu = np.asarray(weight_mu, dtype=f)
    weight_sigma = np.asarray(weight_sigma, dtype=f)
    eps_w = np.asarray(eps_w, dtype=f)
    bias_mu = np.asarray(bias_mu, dtype=f)
    bias_sigma = np.asarray(bias_sigma, dtype=f)
    eps_b = np.asarray(eps_b, dtype=f)

    xb = _pack_x(x, BT, KT)
    in_maps = []
    for o in range(N_CORES):
        osl = slice(o * O_S, (o + 1) * O_S)
        in_maps.append(
            {
                "xt": xb,
                "wpk": _pack_w(weight_sigma[osl], weight_mu[osl], eps_w[osl], KT, O_S),
                "bpk": np.ascontiguousarray(
                    np.stack([bias_sigma[osl], bias_mu[osl], eps_b[osl]])[None]
                ),
            }
        )
    return in_maps


def kernel(x, weight_mu, weight_sigma, bias_mu, bias_sigma, eps_w, eps_b, **run_kwargs):
    from concourse.bass_utils import run_bass_kernel_spmd

    O_S = OUT_FULL // O_SHARDS
    in_maps = _shard_inputs(
        x, weight_mu, weight_sigma, bias_mu, bias_sigma, eps_w, eps_b
    )
    nc = _get_nc()
    res = run_bass_kernel_spmd(nc, in_maps, core_ids=list(range(N_CORES)), **run_kwargs)
    out = np.empty((B_FULL, OUT_FULL), np.float32)
    kl_val = 0.0
    for c in range(N_CORES):
        out[:, c * O_S : (c + 1) * O_S] = res.results[c]["out"]
        kl_val += float(res.results[c]["kl"][0, 0])
    if run_kwargs:
        kernel.last_results = res
    return out, np.float32(kl_val)


# revision 27
# speedup vs baseline: 1.0019x; 1.0019x over previous
"""Bayesian linear layer (reparameterized sample + KL) on 8 Trainium2 NeuronCores.

Reference computation (all fp32):
    weight = weight_mu + exp(weight_sigma) * eps_w          # [OUT, IN]
    bias   = bias_mu   + exp(bias_sigma)   * eps_b          # [OUT]
    out    = x @ weight.T + bias                            # [B, OUT]
    kl     = -0.5 * sum(1 + 2*ws - wm^2 - exp(2*ws))        # over weight
             -0.5 * sum(1 + 2*bs - bm^2 - exp(2*bs))        # over bias

Sharding: column-parallel 1x8.  out_features split in 8 (O_S=512 per core), x
replicated.  Per core: W^T shard [4096, 512] stays resident in SBUF (64KB per
partition, f32r), x streams through in 64 batch tiles.  The weight-param
stream is only 25MB/core so the W-generation phase is short, and 4 batch
tiles run a PE "prologue" during it (k-sliced PSUM accumulation across all 8
banks) so the tensor engine never goes cold.

Host-side layout work (part of the sharding strategy):
  - x is passed pre-transposed AND pre-tiled as [64, 128, 32, 128]
    ([batch-tile, feature-in-ktile, ktile, batch-in-tile]) so each batch-tile
    load is one DMA with 16KB-contiguous per-partition runs.
  - weight mu/sigma/eps shards are interleaved per k-tile as [32, 128, 3, 512]
    (6KB-contiguous per partition) -> one DMA per k-tile.
The contraction dim lands on SBUF partitions with no on-device transposes.

Matmuls run in float32r (the PE's full-rate fp32 mode, tf32-like operand
rounding, fp32 PSUM accumulation).
"""

import sys

import numpy as np

try:
    import concourse.bass as bass  # noqa: F401
except ImportError:  # pragma: no cover
    sys.path.insert(0, "/opt/trn_rl_repo")

import concourse.bass as bass
import concourse.tile as tile
from concourse import bacc, bass_isa, mybir

P = 128
B_FULL, IN_FULL, OUT_FULL = 8192, 4096, 4096
O_SHARDS = 8
N_CORES = 8

F32 = mybir.dt.float32
# fp16 operands: 11-bit significand, the same precision class as the PE's
# tf32-like f32r operand rounding, at half the DMA bytes and SBUF footprint.
X_DT = mybir.dt.float16
W_DT = mybir.dt.float16
MM_DT = X_DT

AF = mybir.ActivationFunctionType
OP = mybir.AluOpType
AX = mybir.AxisListType


def build_bayes_kernel(nc, IN, B_S, O_S, mm_dt=X_DT, w_dt=W_DT):
    """Emit the per-core SPMD program. Tensors are declared on nc."""
    KT = IN // P        # k tiles
    BT = B_S // P       # batch tiles
    assert O_S <= 512   # one psum bank per out tile

    xt = nc.dram_tensor("xt", [BT, P, KT, P], mm_dt, kind="ExternalInput").ap()
    wpk = nc.dram_tensor("wpk", [KT, P, 3, O_S], F32, kind="ExternalInput").ap()
    bpk = nc.dram_tensor("bpk", [1, 3, O_S], F32, kind="ExternalInput").ap()
    out = nc.dram_tensor("out", [B_S, O_S], F32, kind="ExternalOutput").ap()
    kl = nc.dram_tensor("kl", [1, 1], F32, kind="ExternalOutput").ap()

    out_r = out.rearrange("(bt p) o -> p bt o", p=P)

    with tile.TileContext(nc) as tc:
        with (
            tc.tile_pool(name="wpool", bufs=1) as wpool,
            tc.tile_pool(name="gen", bufs=4) as gen,
            tc.tile_pool(name="xin", bufs=10) as xin,
            tc.tile_pool(name="ost", bufs=3) as ost,
            tc.tile_pool(name="misc", bufs=1) as misc,
            tc.tile_pool(name="psum", bufs=8, space="PSUM") as psum,
        ):
            # Persistent state
            w_tiles = [
                wpool.tile([P, O_S], w_dt, tag=f"w{kt}", name=f"w{kt}")
                for kt in range(KT)
            ]
            ssig = misc.tile([P, KT], F32, tag="ssig")   # per-ktile sum(sigma)
            smu2 = misc.tile([P, KT], F32, tag="smu2")   # sum(mu^2)
            sv2 = misc.tile([P, KT], F32, tag="sv2")     # sum(exp(2 sigma))
            b_bc = misc.tile([P, O_S], F32, tag="bbc")   # bias broadcast
            klb = misc.tile([1, 4], F32, tag="klb")      # bias kl: ssig, smu2, sv2, tmp

            # The first W-param tiles are the critical path (the whole kernel
            # waits on W[0]); issue them ahead of the bulk x prefetch, then
            # interleave so the prologue's x tiles still arrive in time.
            PG = min(8, BT)
            gtiles = []
            for kt in range(min(4, KT)):
                g = gen.tile([P, 3, O_S], F32, tag="wpk", name=f"gpre{kt}")
                nc.sync.dma_start(g, wpk[kt])
                gtiles.append(g)
            xpre = []
            for bt in range(PG):
                xs = xin.tile([P, KT, P], mm_dt, tag="xs", name=f"xpre{bt}")
                nc.sync.dma_start(xs, xt[bt])
                xpre.append(xs)
            pg_ps = [
                psum.tile([P, O_S], F32, tag="ps", name=f"pg{pb}")
                for pb in range(PG)
            ]

            # ---- Phase 1: W = mu + exp(sigma)*eps, KL partial sums ----
            # ACT ops depend only on the k-tile's DMA (dumps go to dedicated
            # scratch), DVE work runs back-to-back, and the 4-deep gen pool
            # pipelines the DMA->ACT->DVE chain across k-tiles.  The first PG
            # batch tiles accumulate their matmuls k-tile by k-tile right here
            # (PE prologue) so the PE consumes each W tile as it is produced.
            for kt in range(KT):
                if kt < len(gtiles):
                    g = gtiles[kt]
                else:
                    g = gen.tile([P, 3, O_S], F32, tag="wpk")
                    nc.sync.dma_start(g, wpk[kt])
                sig, mu, eps = g[:, 0, :], g[:, 1, :], g[:, 2, :]
                wtmp = gen.tile([P, O_S], F32, tag="wtmp")
                nc.scalar.activation(wtmp, sig, AF.Exp)             # v
                d2 = misc.tile([P, O_S], F32, tag="sqd")
                nc.scalar.activation(
                    d2, mu, AF.Square, accum_out=smu2[:, kt : kt + 1]
                )
                if kt % 2 == 0:
                    nc.vector.tensor_reduce(
                        ssig[:, kt : kt + 1], sig, AX.X, OP.add
                    )
                else:
                    # balance phase-1 engine load: odd k-tiles sum sigma on ACT
                    d3 = misc.tile([P, O_S], F32, tag="sqd")
                    nc.scalar.activation(
                        d3, sig, AF.Identity, accum_out=ssig[:, kt : kt + 1]
                    )
                # v^2 summed; tensor output dumps over the dead sigma slot
                nc.vector.scalar_tensor_tensor(
                    sig, wtmp, 1.0, wtmp, OP.mult, OP.mult,
                    accum_out=sv2[:, kt : kt + 1],
                )
                nc.vector.tensor_tensor(wtmp, wtmp, eps, OP.mult)
                # final add writes the f32r matmul operand (single rounding)
                nc.vector.tensor_tensor(w_tiles[kt], wtmp, mu, OP.add)
                for pb in range(PG):
                    nc.tensor.matmul(
                        pg_ps[pb],
                        xpre[pb][:, kt, :],
                        w_tiles[kt],
                        start=(kt == 0),
                        stop=(kt == KT - 1),
                    )

            # ---- Bias: value + KL pieces ----
            # all SBUF operands of an op share start partition 0; dead slots
            # of the packed tile double as ACT dump targets.
            bt_ = gen.tile([1, 3, O_S], F32, tag="wpk", name="biastile")
            nc.sync.dma_start(bt_, bpk[0])
            bsig, bmu, beps = bt_[:, 0, :], bt_[:, 1, :], bt_[:, 2, :]
            bv = b_bc[0:1, :]
            nc.vector.tensor_reduce(klb[:, 0:1], bsig, AX.X, OP.add)
            nc.scalar.activation(bv, bsig, AF.Exp)
            nc.vector.tensor_tensor(bv, bv, beps, OP.mult)   # beps dead after
            nc.vector.tensor_tensor(bv, bv, bmu, OP.add)
            nc.scalar.activation(
                beps, bsig, AF.Exp, scale=2.0, accum_out=klb[:, 2:3]
            )
            nc.scalar.activation(bsig, bmu, AF.Square, accum_out=klb[:, 1:2])
            nc.gpsimd.partition_broadcast(b_bc, bv)

            # ---- Prologue eviction: bias-add + store for the PG tiles ----
            for pb in range(PG):
                osb = ost.tile([P, O_S], F32, tag="osb", name=f"osb_pg{pb}")
                nc.vector.tensor_tensor(osb, pg_ps[pb], b_bc, OP.add)
                nc.sync.dma_start(out_r[:, pb, :], osb)

            # ---- Phase 2: out[bt] = x[bt] @ W^T + bias ----
            for bt in range(PG, BT):
                xs = xin.tile([P, KT, P], mm_dt, tag="xs")
                nc.sync.dma_start(xs, xt[bt])
                ps = psum.tile([P, O_S], F32, tag="ps")
                for kt in range(KT):
                    nc.tensor.matmul(
                        ps,
                        xs[:, kt, :],
                        w_tiles[kt],
                        start=(kt == 0),
                        stop=(kt == KT - 1),
                    )
                osb = ost.tile([P, O_S], F32, tag="osb")
                nc.vector.tensor_tensor(osb, ps, b_bc, OP.add)
                nc.sync.dma_start(out_r[:, bt, :], osb)

            # ---- KL tail ----
            rs = misc.tile([P, 1], F32, tag="rs")
            rm = misc.tile([P, 1], F32, tag="rm")
            rv = misc.tile([P, 1], F32, tag="rv")
            nc.vector.tensor_reduce(rs, ssig, AX.X, OP.add)
            nc.vector.tensor_reduce(rm, smu2, AX.X, OP.add)
            nc.vector.tensor_reduce(rv, sv2, AX.X, OP.add)
            tcol = misc.tile([P, 1], F32, tag="tcol")
            # tcol = 2*rs - rm - rv
            nc.vector.scalar_tensor_tensor(tcol, rs, 2.0, rm, OP.mult, OP.subtract)
            nc.vector.tensor_tensor(tcol, tcol, rv, OP.subtract)
            # bias terms fold into partition 0
            nc.vector.scalar_tensor_tensor(
                klb[:, 3:4], klb[:, 0:1], 2.0, klb[:, 1:2], OP.mult, OP.subtract
            )
            nc.vector.tensor_tensor(klb[:, 3:4], klb[:, 3:4], klb[:, 2:3], OP.subtract)
            nc.vector.tensor_tensor(tcol[0:1, :], tcol[0:1, :], klb[:, 3:4], OP.add)
            tall = misc.tile([P, 1], F32, tag="tall")
            nc.gpsimd.partition_all_reduce(tall, tcol, P, bass_isa.ReduceOp.add)
            # kl = -0.5 * (count + sum(2s - m^2 - v^2))
            count = float(IN * O_S + O_S)
            klt = misc.tile([1, 1], F32, tag="klt")
            nc.vector.tensor_scalar(klt, tall[0:1, :], count, -0.5, OP.add, OP.mult)
            nc.sync.dma_start(kl, klt)


_NC_CACHE = {}


def _get_nc():
    key = "full"
    if key not in _NC_CACHE:
        nc = bacc.Bacc("TRN2", target_bir_lowering=False, debug=False)
        build_bayes_kernel(nc, IN_FULL, B_FULL, OUT_FULL // O_SHARDS)
        nc.compile()
        _NC_CACHE[key] = nc
    return _NC_CACHE[key]


def _pack_x(x_full, BT, KT):
    # [B, IN] -> [BT, P, KT, P] with [bt, p(=feature in tile), kt, b], fp16
    x4 = x_full.reshape(BT, P, KT, P)           # [bt, b, kt, p_feature]
    return np.ascontiguousarray(x4.transpose(0, 3, 2, 1).astype(np.float16))


def _pack_w(sig, mu, eps, KT, O_S):
    # each [O_S, IN] -> packed [KT, P, 3, O_S] with feature on partition
    stk = np.stack([sig.T, mu.T, eps.T], axis=1)   # [IN, 3, O_S]
    return np.ascontiguousarray(stk.reshape(KT, P, 3, O_S))


def _shard_inputs(x, weight_mu, weight_sigma, bias_mu, bias_sigma, eps_w, eps_b):
    O_S = OUT_FULL // O_SHARDS
    BT, KT = B_FULL // P, IN_FULL // P
    f = np.float32
    x = np.asarray(x, dtype=f)
    weight_mu = np.asarray(weight_mu, dtype=f)
    weight_sigma = np.asarray(weight_sigma, dtype=f)
    eps_w = np.asarray(eps_w, dtype=f)
    bias_mu = np.asarray(bias_mu, dtype=f)
    bias_sigma = np.asarray(bias_sigma, dtype=f)
    eps_b = np.asarray(eps_b, dtype=f)

    xb = _pack_x(x, BT, KT)
    in_maps = []
    for o in range(N_CORES):
        osl = slice(o * O_S, (o + 1) * O_S)
        in_maps.append(
            {
                "xt": xb,
                "wpk": _pack_w(weight_sigma[osl], weight_mu[osl], eps_w[osl], KT, O_S),
                "bpk": np.ascontiguousarray(
                    np.stack([bias_sigma[osl], bias_mu[osl], eps_b[osl]])[None]
                ),
            }
        )
    return in_maps


def kernel(x, weight_mu, weight_sigma, bias_mu, bias_sigma, eps_w, eps_b, **run_kwargs):
    from concourse.bass_utils import run_bass_kernel_spmd

    O_S = OUT_FULL // O_SHARDS
    in_maps = _shard_inputs(
        x, weight_mu, weight_sigma, bias_mu, bias_sigma, eps_w, eps_b
    )
    nc = _get_nc()
    res = run_bass_kernel_spmd(nc, in_maps, core_ids=list(range(N_CORES)), **run_kwargs)
    out = np.empty((B_FULL, OUT_FULL), np.float32)
    kl_val = 0.0
    for c in range(N_CORES):
        out[:, c * O_S : (c + 1) * O_S] = res.results[c]["out"]
        kl_val += float(res.results[c]["kl"][0, 0])
    if run_kwargs:
        kernel.last_results = res
    return out, np.float32(kl_val)


# revision 32
# speedup vs baseline: 1.0714x; 1.0693x over previous
"""Bayesian linear layer (reparameterized sample + KL) on 8 Trainium2 NeuronCores.

Reference computation (all fp32):
    weight = weight_mu + exp(weight_sigma) * eps_w          # [OUT, IN]
    bias   = bias_mu   + exp(bias_sigma)   * eps_b          # [OUT]
    out    = x @ weight.T + bias                            # [B, OUT]
    kl     = -0.5 * sum(1 + 2*ws - wm^2 - exp(2*ws))        # over weight
             -0.5 * sum(1 + 2*bs - bm^2 - exp(2*bs))        # over bias

Sharding: column-parallel 1x8.  out_features split in 8 (O_S=512 per core), x
replicated.  Per core: W^T shard [4096, 512] stays resident in SBUF (64KB per
partition, f32r), x streams through in 64 batch tiles.  The weight-param
stream is only 25MB/core so the W-generation phase is short, and 4 batch
tiles run a PE "prologue" during it (k-sliced PSUM accumulation across all 8
banks) so the tensor engine never goes cold.

Host-side layout work (part of the sharding strategy):
  - x is passed pre-transposed AND pre-tiled as [64, 128, 32, 128]
    ([batch-tile, feature-in-ktile, ktile, batch-in-tile]) so each batch-tile
    load is one DMA with 16KB-contiguous per-partition runs.
  - weight mu/sigma/eps shards are interleaved per k-tile as [32, 128, 3, 512]
    (6KB-contiguous per partition) -> one DMA per k-tile.
The contraction dim lands on SBUF partitions with no on-device transposes.

Matmuls run in float32r (the PE's full-rate fp32 mode, tf32-like operand
rounding, fp32 PSUM accumulation).
"""

import sys

import numpy as np

try:
    import concourse.bass as bass  # noqa: F401
except ImportError:  # pragma: no cover
    sys.path.insert(0, "/opt/trn_rl_repo")

import concourse.bass as bass
import concourse.tile as tile
from concourse import bacc, bass_isa, mybir

P = 128
B_FULL, IN_FULL, OUT_FULL = 8192, 4096, 4096
O_SHARDS = 8
N_CORES = 8

F32 = mybir.dt.float32
# fp16 operands: 11-bit significand, the same precision class as the PE's
# tf32-like f32r operand rounding, at half the DMA bytes and SBUF footprint.
X_DT = mybir.dt.float16
W_DT = mybir.dt.float16
MM_DT = X_DT

AF = mybir.ActivationFunctionType
OP = mybir.AluOpType
AX = mybir.AxisListType


def build_bayes_kernel(nc, IN, B_S, O_S, mm_dt=X_DT, w_dt=W_DT):
    """Emit the per-core SPMD program. Tensors are declared on nc."""
    KT = IN // P        # k tiles
    BT = B_S // P       # batch tiles
    assert O_S <= 512   # one psum bank per out tile

    xt = nc.dram_tensor("xt", [BT, P, KT, P], mm_dt, kind="ExternalInput").ap()
    # sigma stays fp32 (it feeds exp and the KL sums); mu/eps stream as fp16 —
    # their rounding is absorbed by the fp16 rounding of W itself.
    wsg = nc.dram_tensor("wsg", [KT, P, O_S], F32, kind="ExternalInput").ap()
    wme = nc.dram_tensor("wme", [KT, P, 2, O_S], mm_dt, kind="ExternalInput").ap()
    bpk = nc.dram_tensor("bpk", [1, 3, O_S], F32, kind="ExternalInput").ap()
    out = nc.dram_tensor("out", [B_S, O_S], F32, kind="ExternalOutput").ap()
    kl = nc.dram_tensor("kl", [1, 1], F32, kind="ExternalOutput").ap()

    out_r = out.rearrange("(bt p) o -> p bt o", p=P)

    with tile.TileContext(nc) as tc:
        with (
            tc.tile_pool(name="wpool", bufs=1) as wpool,
            tc.tile_pool(name="gen", bufs=4) as gen,
            tc.tile_pool(name="xin", bufs=10) as xin,
            tc.tile_pool(name="ost", bufs=3) as ost,
            tc.tile_pool(name="misc", bufs=1) as misc,
            tc.tile_pool(name="psum", bufs=8, space="PSUM") as psum,
        ):
            # Persistent state
            w_tiles = [
                wpool.tile([P, O_S], w_dt, tag=f"w{kt}", name=f"w{kt}")
                for kt in range(KT)
            ]
            ssig = misc.tile([P, KT], F32, tag="ssig")   # per-ktile sum(sigma)
            smu2 = misc.tile([P, KT], F32, tag="smu2")   # sum(mu^2)
            sv2 = misc.tile([P, KT], F32, tag="sv2")     # sum(exp(2 sigma))
            b_bc = misc.tile([P, O_S], F32, tag="bbc")   # bias broadcast
            klb = misc.tile([1, 4], F32, tag="klb")      # bias kl: ssig, smu2, sv2, tmp

            # The first W-param tiles are the critical path (the whole kernel
            # waits on W[0]); issue them ahead of everything, then let the x
            # prefetches interleave with the remaining W stream inside the
            # loop (xpre[i]'s DMA is issued a couple of k-steps before the
            # skewed prologue first reads it).
            PG = min(8, BT)
            NPRE = min(4, KT)
            gs_pre, gm_pre = [], []
            for kt in range(NPRE):
                gs = gen.tile([P, O_S], F32, tag="wsg", name=f"gspre{kt}")
                nc.sync.dma_start(gs, wsg[kt])
                gs_pre.append(gs)
                gm = gen.tile([P, 2, O_S], mm_dt, tag="wme", name=f"gmpre{kt}")
                nc.sync.dma_start(gm, wme[kt])
                gm_pre.append(gm)
            xpre = [
                xin.tile([P, KT, P], mm_dt, tag="xs", name=f"xpre{bt}")
                for bt in range(PG)
            ]
            xpre_issued = 0

            def _issue_xpre(n):
                nonlocal xpre_issued
                while xpre_issued < min(n, PG):
                    nc.sync.dma_start(xpre[xpre_issued], xt[xpre_issued])
                    xpre_issued += 1

            _issue_xpre(2)
            pg_ps = [
                psum.tile([P, O_S], F32, tag="ps", name=f"pg{pb}")
                for pb in range(PG)
            ]

            def _prologue_mm(pb, kt2):
                nc.tensor.matmul(
                    pg_ps[pb],
                    xpre[pb][:, kt2, :],
                    w_tiles[kt2],
                    start=(kt2 == 0),
                    stop=(kt2 == KT - 1),
                )

            # ---- Phase 1: W = mu + exp(sigma)*eps, KL partial sums ----
            # ACT ops depend only on the k-tile's DMA (dumps go to dedicated
            # scratch), DVE work runs back-to-back, and the 4-deep gen pool
            # pipelines the DMA->ACT->DVE chain across k-tiles.  The first PG
            # batch tiles accumulate their matmuls k-tile by k-tile right here
            # (PE prologue), skewed diagonally (tile pb trails by pb k-steps)
            # so each prefetched x tile is needed only after its DMA lands.
            for kt in range(KT):
                if kt < NPRE:
                    gs, gm = gs_pre[kt], gm_pre[kt]
                else:
                    gs = gen.tile([P, O_S], F32, tag="wsg")
                    nc.sync.dma_start(gs, wsg[kt])
                    gm = gen.tile([P, 2, O_S], mm_dt, tag="wme")
                    nc.sync.dma_start(gm, wme[kt])
                _issue_xpre(kt + 3)
                mu, eps = gm[:, 0, :], gm[:, 1, :]
                wtmp = gen.tile([P, O_S], F32, tag="wtmp")
                nc.scalar.activation(wtmp, gs, AF.Exp)             # v
                d2 = misc.tile([P, O_S], F32, tag="sqd")
                nc.scalar.activation(
                    d2, mu, AF.Square, accum_out=smu2[:, kt : kt + 1]
                )
                if kt % 2 == 0:
                    nc.vector.tensor_reduce(
                        ssig[:, kt : kt + 1], gs, AX.X, OP.add
                    )
                else:
                    # balance phase-1 engine load: odd k-tiles sum sigma on ACT
                    d3 = misc.tile([P, O_S], F32, tag="sqd")
                    nc.scalar.activation(
                        d3, gs, AF.Identity, accum_out=ssig[:, kt : kt + 1]
                    )
                # v^2 summed; tensor output dumps over the dead sigma slot
                nc.vector.scalar_tensor_tensor(
                    gs, wtmp, 1.0, wtmp, OP.mult, OP.mult,
                    accum_out=sv2[:, kt : kt + 1],
                )
                nc.vector.tensor_tensor(wtmp, wtmp, eps, OP.mult)
                # final add writes the fp16 matmul operand (single rounding)
                nc.vector.tensor_tensor(w_tiles[kt], wtmp, mu, OP.add)
                for pb in range(PG):
                    kt2 = kt - pb
                    if kt2 >= 0:
                        _prologue_mm(pb, kt2)
            # skew tail: the trailing k-tiles of prologue tiles 1..PG-1
            for pb in range(1, PG):
                for kt2 in range(max(KT - pb, 0), KT):
                    _prologue_mm(pb, kt2)

            # ---- Bias: value + KL pieces ----
            # all SBUF operands of an op share start partition 0; dead slots
            # of the packed tile double as ACT dump targets.
            bt_ = misc.tile([1, 3, O_S], F32, tag="biastile", name="biastile")
            nc.sync.dma_start(bt_, bpk[0])
            bsig, bmu, beps = bt_[:, 0, :], bt_[:, 1, :], bt_[:, 2, :]
            bv = b_bc[0:1, :]
            nc.vector.tensor_reduce(klb[:, 0:1], bsig, AX.X, OP.add)
            nc.scalar.activation(bv, bsig, AF.Exp)
            nc.vector.tensor_tensor(bv, bv, beps, OP.mult)   # beps dead after
            nc.vector.tensor_tensor(bv, bv, bmu, OP.add)
            nc.scalar.activation(
                beps, bsig, AF.Exp, scale=2.0, accum_out=klb[:, 2:3]
            )
            nc.scalar.activation(bsig, bmu, AF.Square, accum_out=klb[:, 1:2])
            nc.gpsimd.partition_broadcast(b_bc, bv)

            # ---- Prologue eviction: bias-add + store for the PG tiles ----
            for pb in range(PG):
                osb = ost.tile([P, O_S], F32, tag="osb", name=f"osb_pg{pb}")
                nc.vector.tensor_tensor(osb, pg_ps[pb], b_bc, OP.add)
                nc.sync.dma_start(out_r[:, pb, :], osb)

            # ---- Phase 2: out[bt] = x[bt] @ W^T + bias ----
            for bt in range(PG, BT):
                xs = xin.tile([P, KT, P], mm_dt, tag="xs")
                nc.sync.dma_start(xs, xt[bt])
                ps = psum.tile([P, O_S], F32, tag="ps")
                for kt in range(KT):
                    nc.tensor.matmul(
                        ps,
                        xs[:, kt, :],
                        w_tiles[kt],
                        start=(kt == 0),
                        stop=(kt == KT - 1),
                    )
                osb = ost.tile([P, O_S], F32, tag="osb")
                nc.vector.tensor_tensor(osb, ps, b_bc, OP.add)
                nc.sync.dma_start(out_r[:, bt, :], osb)

            # ---- KL tail ----
            rs = misc.tile([P, 1], F32, tag="rs")
            rm = misc.tile([P, 1], F32, tag="rm")
            rv = misc.tile([P, 1], F32, tag="rv")
            nc.vector.tensor_reduce(rs, ssig, AX.X, OP.add)
            nc.vector.tensor_reduce(rm, smu2, AX.X, OP.add)
            nc.vector.tensor_reduce(rv, sv2, AX.X, OP.add)
            tcol = misc.tile([P, 1], F32, tag="tcol")
            # tcol = 2*rs - rm - rv
            nc.vector.scalar_tensor_tensor(tcol, rs, 2.0, rm, OP.mult, OP.subtract)
            nc.vector.tensor_tensor(tcol, tcol, rv, OP.subtract)
            # bias terms fold into partition 0
            nc.vector.scalar_tensor_tensor(
                klb[:, 3:4], klb[:, 0:1], 2.0, klb[:, 1:2], OP.mult, OP.subtract
            )
            nc.vector.tensor_tensor(klb[:, 3:4], klb[:, 3:4], klb[:, 2:3], OP.subtract)
            nc.vector.tensor_tensor(tcol[0:1, :], tcol[0:1, :], klb[:, 3:4], OP.add)
            tall = misc.tile([P, 1], F32, tag="tall")
            nc.gpsimd.partition_all_reduce(tall, tcol, P, bass_isa.ReduceOp.add)
            # kl = -0.5 * (count + sum(2s - m^2 - v^2))
            count = float(IN * O_S + O_S)
            klt = misc.tile([1, 1], F32, tag="klt")
            nc.vector.tensor_scalar(klt, tall[0:1, :], count, -0.5, OP.add, OP.mult)
            nc.sync.dma_start(kl, klt)


_NC_CACHE = {}


def _get_nc():
    key = "full"
    if key not in _NC_CACHE:
        nc = bacc.Bacc("TRN2", target_bir_lowering=False, debug=False)
        build_bayes_kernel(nc, IN_FULL, B_FULL, OUT_FULL // O_SHARDS)
        nc.compile()
        _NC_CACHE[key] = nc
    return _NC_CACHE[key]


def _pack_x(x_full, BT, KT):
    # [B, IN] -> [BT, P, KT, P] with [bt, p(=feature in tile), kt, b], fp16
    x4 = x_full.reshape(BT, P, KT, P)           # [bt, b, kt, p_feature]
    return np.ascontiguousarray(x4.transpose(0, 3, 2, 1).astype(np.float16))


def _pack_w(sig, mu, eps, KT, O_S):
    # sigma: [O_S, IN] -> [KT, P, O_S] fp32; mu/eps -> [KT, P, 2, O_S] fp16
    wsg = np.ascontiguousarray(sig.T.reshape(KT, P, O_S))
    stk = np.stack([mu.T, eps.T], axis=1).astype(np.float16)   # [IN, 2, O_S]
    wme = np.ascontiguousarray(stk.reshape(KT, P, 2, O_S))
    return wsg, wme


def _shard_inputs(x, weight_mu, weight_sigma, bias_mu, bias_sigma, eps_w, eps_b):
    O_S = OUT_FULL // O_SHARDS
    BT, KT = B_FULL // P, IN_FULL // P
    f = np.float32
    x = np.asarray(x, dtype=f)
    weight_mu = np.asarray(weight_mu, dtype=f)
    weight_sigma = np.asarray(weight_sigma, dtype=f)
    eps_w = np.asarray(eps_w, dtype=f)
    bias_mu = np.asarray(bias_mu, dtype=f)
    bias_sigma = np.asarray(bias_sigma, dtype=f)
    eps_b = np.asarray(eps_b, dtype=f)

    xb = _pack_x(x, BT, KT)
    in_maps = []
    for o in range(N_CORES):
        osl = slice(o * O_S, (o + 1) * O_S)
        wsg, wme = _pack_w(weight_sigma[osl], weight_mu[osl], eps_w[osl], KT, O_S)
        in_maps.append(
            {
                "xt": xb,
                "wsg": wsg,
                "wme": wme,
                "bpk": np.ascontiguousarray(
                    np.stack([bias_sigma[osl], bias_mu[osl], eps_b[osl]])[None]
                ),
            }
        )
    return in_maps


def kernel(x, weight_mu, weight_sigma, bias_mu, bias_sigma, eps_w, eps_b, **run_kwargs):
    from concourse.bass_utils import run_bass_kernel_spmd

    O_S = OUT_FULL // O_SHARDS
    in_maps = _shard_inputs(
        x, weight_mu, weight_sigma, bias_mu, bias_sigma, eps_w, eps_b
    )
    nc = _get_nc()
    res = run_bass_kernel_spmd(nc, in_maps, core_ids=list(range(N_CORES)), **run_kwargs)
    out = np.empty((B_FULL, OUT_FULL), np.float32)
    kl_val = 0.0
    for c in range(N_CORES):
        out[:, c * O_S : (c + 1) * O_S] = res.results[c]["out"]
        kl_val += float(res.results[c]["kl"][0, 0])
    if run_kwargs:
        kernel.last_results = res
    return out, np.float32(kl_val)


# revision 33
# speedup vs baseline: 1.0760x; 1.0043x over previous
"""Bayesian linear layer (reparameterized sample + KL) on 8 Trainium2 NeuronCores.

Reference computation (all fp32):
    weight = weight_mu + exp(weight_sigma) * eps_w          # [OUT, IN]
    bias   = bias_mu   + exp(bias_sigma)   * eps_b          # [OUT]
    out    = x @ weight.T + bias                            # [B, OUT]
    kl     = -0.5 * sum(1 + 2*ws - wm^2 - exp(2*ws))        # over weight
             -0.5 * sum(1 + 2*bs - bm^2 - exp(2*bs))        # over bias

Sharding: column-parallel 1x8.  out_features split in 8 (O_S=512 per core), x
replicated.  Per core: W^T shard [4096, 512] stays resident in SBUF (64KB per
partition, f32r), x streams through in 64 batch tiles.  The weight-param
stream is only 25MB/core so the W-generation phase is short, and 4 batch
tiles run a PE "prologue" during it (k-sliced PSUM accumulation across all 8
banks) so the tensor engine never goes cold.

Host-side layout work (part of the sharding strategy):
  - x is passed pre-transposed AND pre-tiled as [64, 128, 32, 128]
    ([batch-tile, feature-in-ktile, ktile, batch-in-tile]) so each batch-tile
    load is one DMA with 16KB-contiguous per-partition runs.
  - weight mu/sigma/eps shards are interleaved per k-tile as [32, 128, 3, 512]
    (6KB-contiguous per partition) -> one DMA per k-tile.
The contraction dim lands on SBUF partitions with no on-device transposes.

Matmuls run in float32r (the PE's full-rate fp32 mode, tf32-like operand
rounding, fp32 PSUM accumulation).
"""

import sys

import numpy as np

try:
    import concourse.bass as bass  # noqa: F401
except ImportError:  # pragma: no cover
    sys.path.insert(0, "/opt/trn_rl_repo")

import concourse.bass as bass
import concourse.tile as tile
from concourse import bacc, bass_isa, mybir

P = 128
B_FULL, IN_FULL, OUT_FULL = 8192, 4096, 4096
O_SHARDS = 8
N_CORES = 8

F32 = mybir.dt.float32
# fp16 operands: 11-bit significand, the same precision class as the PE's
# tf32-like f32r operand rounding, at half the DMA bytes and SBUF footprint.
X_DT = mybir.dt.float16
W_DT = mybir.dt.float16
MM_DT = X_DT

AF = mybir.ActivationFunctionType
OP = mybir.AluOpType
AX = mybir.AxisListType


def build_bayes_kernel(nc, IN, B_S, O_S, mm_dt=X_DT, w_dt=W_DT):
    """Emit the per-core SPMD program. Tensors are declared on nc."""
    KT = IN // P        # k tiles
    BT = B_S // P       # batch tiles
    assert O_S <= 512   # one psum bank per out tile

    xt = nc.dram_tensor("xt", [BT, P, KT, P], mm_dt, kind="ExternalInput").ap()
    # sigma stays fp32 (it feeds exp and the KL sums); mu/eps stream as fp16 —
    # their rounding is absorbed by the fp16 rounding of W itself.
    wsg = nc.dram_tensor("wsg", [KT, P, O_S], F32, kind="ExternalInput").ap()
    wme = nc.dram_tensor("wme", [KT, P, 2, O_S], mm_dt, kind="ExternalInput").ap()
    bpk = nc.dram_tensor("bpk", [1, 3, O_S], F32, kind="ExternalInput").ap()
    out = nc.dram_tensor("out", [B_S, O_S], F32, kind="ExternalOutput").ap()
    kl = nc.dram_tensor("kl", [1, 1], F32, kind="ExternalOutput").ap()

    out_r = out.rearrange("(bt p) o -> p bt o", p=P)

    with tile.TileContext(nc) as tc:
        with (
            tc.tile_pool(name="wpool", bufs=1) as wpool,
            tc.tile_pool(name="gen", bufs=6) as gen,
            tc.tile_pool(name="xin", bufs=12) as xin,
            tc.tile_pool(name="ost", bufs=3) as ost,
            tc.tile_pool(name="misc", bufs=1) as misc,
            tc.tile_pool(name="psum", bufs=8, space="PSUM") as psum,
        ):
            # Persistent state
            w_tiles = [
                wpool.tile([P, O_S], w_dt, tag=f"w{kt}", name=f"w{kt}")
                for kt in range(KT)
            ]
            ssig = misc.tile([P, KT], F32, tag="ssig")   # per-ktile sum(sigma)
            smu2 = misc.tile([P, KT], F32, tag="smu2")   # sum(mu^2)
            sv2 = misc.tile([P, KT], F32, tag="sv2")     # sum(exp(2 sigma))
            b_bc = misc.tile([P, O_S], F32, tag="bbc")   # bias broadcast
            klb = misc.tile([1, 4], F32, tag="klb")      # bias kl: ssig, smu2, sv2, tmp

            # The first W-param tiles are the critical path (the whole kernel
            # waits on W[0]); issue them ahead of everything, then let the x
            # prefetches interleave with the remaining W stream inside the
            # loop (xpre[i]'s DMA is issued a couple of k-steps before the
            # skewed prologue first reads it).
            PG = min(8, BT)
            NPRE = min(6, KT)
            gs_pre, gm_pre = [], []
            for kt in range(NPRE):
                gs = gen.tile([P, O_S], F32, tag="wsg", name=f"gspre{kt}")
                nc.sync.dma_start(gs, wsg[kt])
                gs_pre.append(gs)
                gm = gen.tile([P, 2, O_S], mm_dt, tag="wme", name=f"gmpre{kt}")
                nc.sync.dma_start(gm, wme[kt])
                gm_pre.append(gm)
            xpre = [
                xin.tile([P, KT, P], mm_dt, tag="xs", name=f"xpre{bt}")
                for bt in range(PG)
            ]
            xpre_issued = 0

            def _issue_xpre(n):
                nonlocal xpre_issued
                while xpre_issued < min(n, PG):
                    nc.sync.dma_start(xpre[xpre_issued], xt[xpre_issued])
                    xpre_issued += 1

            _issue_xpre(2)
            pg_ps = [
                psum.tile([P, O_S], F32, tag="ps", name=f"pg{pb}")
                for pb in range(PG)
            ]

            def _prologue_mm(pb, kt2):
                nc.tensor.matmul(
                    pg_ps[pb],
                    xpre[pb][:, kt2, :],
                    w_tiles[kt2],
                    start=(kt2 == 0),
                    stop=(kt2 == KT - 1),
                )

            # ---- Phase 1: W = mu + exp(sigma)*eps, KL partial sums ----
            # ACT ops depend only on the k-tile's DMA (dumps go to dedicated
            # scratch), DVE work runs back-to-back, and the 4-deep gen pool
            # pipelines the DMA->ACT->DVE chain across k-tiles.  The first PG
            # batch tiles accumulate their matmuls k-tile by k-tile right here
            # (PE prologue), skewed diagonally (tile pb trails by pb k-steps)
            # so each prefetched x tile is needed only after its DMA lands.
            for kt in range(KT):
                if kt < NPRE:
                    gs, gm = gs_pre[kt], gm_pre[kt]
                else:
                    gs = gen.tile([P, O_S], F32, tag="wsg")
                    nc.sync.dma_start(gs, wsg[kt])
                    gm = gen.tile([P, 2, O_S], mm_dt, tag="wme")
                    nc.sync.dma_start(gm, wme[kt])
                _issue_xpre(kt + 3)
                mu, eps = gm[:, 0, :], gm[:, 1, :]
                wtmp = gen.tile([P, O_S], F32, tag="wtmp")
                nc.scalar.activation(wtmp, gs, AF.Exp)             # v
                d2 = misc.tile([P, O_S], F32, tag="sqd")
                nc.scalar.activation(
                    d2, mu, AF.Square, accum_out=smu2[:, kt : kt + 1]
                )
                if kt % 2 == 0:
                    nc.vector.tensor_reduce(
                        ssig[:, kt : kt + 1], gs, AX.X, OP.add
                    )
                else:
                    # balance phase-1 engine load: odd k-tiles sum sigma on ACT
                    d3 = misc.tile([P, O_S], F32, tag="sqd")
                    nc.scalar.activation(
                        d3, gs, AF.Identity, accum_out=ssig[:, kt : kt + 1]
                    )
                # v^2 summed; tensor output dumps over the dead sigma slot
                nc.vector.scalar_tensor_tensor(
                    gs, wtmp, 1.0, wtmp, OP.mult, OP.mult,
                    accum_out=sv2[:, kt : kt + 1],
                )
                nc.vector.tensor_tensor(wtmp, wtmp, eps, OP.mult)
                # final add writes the fp16 matmul operand (single rounding)
                nc.vector.tensor_tensor(w_tiles[kt], wtmp, mu, OP.add)
                for pb in range(PG):
                    kt2 = kt - pb
                    if kt2 >= 0:
                        _prologue_mm(pb, kt2)
            # skew tail: the trailing k-tiles of prologue tiles 1..PG-1
            for pb in range(1, PG):
                for kt2 in range(max(KT - pb, 0), KT):
                    _prologue_mm(pb, kt2)

            # ---- Bias: value + KL pieces ----
            # all SBUF operands of an op share start partition 0; dead slots
            # of the packed tile double as ACT dump targets.
            bt_ = misc.tile([1, 3, O_S], F32, tag="biastile", name="biastile")
            nc.sync.dma_start(bt_, bpk[0])
            bsig, bmu, beps = bt_[:, 0, :], bt_[:, 1, :], bt_[:, 2, :]
            bv = b_bc[0:1, :]
            nc.vector.tensor_reduce(klb[:, 0:1], bsig, AX.X, OP.add)
            nc.scalar.activation(bv, bsig, AF.Exp)
            nc.vector.tensor_tensor(bv, bv, beps, OP.mult)   # beps dead after
            nc.vector.tensor_tensor(bv, bv, bmu, OP.add)
            nc.scalar.activation(
                beps, bsig, AF.Exp, scale=2.0, accum_out=klb[:, 2:3]
            )
            nc.scalar.activation(bsig, bmu, AF.Square, accum_out=klb[:, 1:2])
            nc.gpsimd.partition_broadcast(b_bc, bv)

            # ---- Prologue eviction: bias-add + store for the PG tiles ----
            for pb in range(PG):
                osb = ost.tile([P, O_S], F32, tag="osb", name=f"osb_pg{pb}")
                nc.vector.tensor_tensor(osb, pg_ps[pb], b_bc, OP.add)
                nc.sync.dma_start(out_r[:, pb, :], osb)

            # ---- Phase 2: out[bt] = x[bt] @ W^T + bias ----
            for bt in range(PG, BT):
                xs = xin.tile([P, KT, P], mm_dt, tag="xs")
                nc.sync.dma_start(xs, xt[bt])
                ps = psum.tile([P, O_S], F32, tag="ps")
                for kt in range(KT):
                    nc.tensor.matmul(
                        ps,
                        xs[:, kt, :],
                        w_tiles[kt],
                        start=(kt == 0),
                        stop=(kt == KT - 1),
                    )
                osb = ost.tile([P, O_S], F32, tag="osb")
                nc.vector.tensor_tensor(osb, ps, b_bc, OP.add)
                nc.sync.dma_start(out_r[:, bt, :], osb)

            # ---- KL tail ----
            rs = misc.tile([P, 1], F32, tag="rs")
            rm = misc.tile([P, 1], F32, tag="rm")
            rv = misc.tile([P, 1], F32, tag="rv")
            nc.vector.tensor_reduce(rs, ssig, AX.X, OP.add)
            nc.vector.tensor_reduce(rm, smu2, AX.X, OP.add)
            nc.vector.tensor_reduce(rv, sv2, AX.X, OP.add)
            tcol = misc.tile([P, 1], F32, tag="tcol")
            # tcol = 2*rs - rm - rv
            nc.vector.scalar_tensor_tensor(tcol, rs, 2.0, rm, OP.mult, OP.subtract)
            nc.vector.tensor_tensor(tcol, tcol, rv, OP.subtract)
            # bias terms fold into partition 0
            nc.vector.scalar_tensor_tensor(
                klb[:, 3:4], klb[:, 0:1], 2.0, klb[:, 1:2], OP.mult, OP.subtract
            )
            nc.vector.tensor_tensor(klb[:, 3:4], klb[:, 3:4], klb[:, 2:3], OP.subtract)
            nc.vector.tensor_tensor(tcol[0:1, :], tcol[0:1, :], klb[:, 3:4], OP.add)
            tall = misc.tile([P, 1], F32, tag="tall")
            nc.gpsimd.partition_all_reduce(tall, tcol, P, bass_isa.ReduceOp.add)
            # kl = -0.5 * (count + sum(2s - m^2 - v^2))
            count = float(IN * O_S + O_S)
            klt = misc.tile([1, 1], F32, tag="klt")
            nc.vector.tensor_scalar(klt, tall[0:1, :], count, -0.5, OP.add, OP.mult)
            nc.sync.dma_start(kl, klt)


_NC_CACHE = {}


def _get_nc():
    key = "full"
    if key not in _NC_CACHE:
        nc = bacc.Bacc("TRN2", target_bir_lowering=False, debug=False)
        build_bayes_kernel(nc, IN_FULL, B_FULL, OUT_FULL // O_SHARDS)
        nc.compile()
        _NC_CACHE[key] = nc
    return _NC_CACHE[key]


def _pack_x(x_full, BT, KT):
    # [B, IN] -> [BT, P, KT, P] with [bt, p(=feature in tile), kt, b], fp16
    x4 = x_full.reshape(BT, P, KT, P)           # [bt, b, kt, p_feature]
    return np.ascontiguousarray(x4.transpose(0, 3, 2, 1).astype(np.float16))


def _pack_w(sig, mu, eps, KT, O_S):
    # sigma: [O_S, IN] -> [KT, P, O_S] fp32; mu/eps -> [KT, P, 2, O_S] fp16
    wsg = np.ascontiguousarray(sig.T.reshape(KT, P, O_S))
    stk = np.stack([mu.T, eps.T], axis=1).astype(np.float16)   # [IN, 2, O_S]
    wme = np.ascontiguousarray(stk.reshape(KT, P, 2, O_S))
    return wsg, wme


def _shard_inputs(x, weight_mu, weight_sigma, bias_mu, bias_sigma, eps_w, eps_b):
    O_S = OUT_FULL // O_SHARDS
    BT, KT = B_FULL // P, IN_FULL // P
    f = np.float32
    x = np.asarray(x, dtype=f)
    weight_mu = np.asarray(weight_mu, dtype=f)
    weight_sigma = np.asarray(weight_sigma, dtype=f)
    eps_w = np.asarray(eps_w, dtype=f)
    bias_mu = np.asarray(bias_mu, dtype=f)
    bias_sigma = np.asarray(bias_sigma, dtype=f)
    eps_b = np.asarray(eps_b, dtype=f)

    xb = _pack_x(x, BT, KT)
    in_maps = []
    for o in range(N_CORES):
        osl = slice(o * O_S, (o + 1) * O_S)
        wsg, wme = _pack_w(weight_sigma[osl], weight_mu[osl], eps_w[osl], KT, O_S)
        in_maps.append(
            {
                "xt": xb,
                "wsg": wsg,
                "wme": wme,
                "bpk": np.ascontiguousarray(
                    np.stack([bias_sigma[osl], bias_mu[osl], eps_b[osl]])[None]
                ),
            }
        )
    return in_maps


def kernel(x, weight_mu, weight_sigma, bias_mu, bias_sigma, eps_w, eps_b, **run_kwargs):
    from concourse.bass_utils import run_bass_kernel_spmd

    O_S = OUT_FULL // O_SHARDS
    in_maps = _shard_inputs(
        x, weight_mu, weight_sigma, bias_mu, bias_sigma, eps_w, eps_b
    )
    nc = _get_nc()
    res = run_bass_kernel_spmd(nc, in_maps, core_ids=list(range(N_CORES)), **run_kwargs)
    out = np.empty((B_FULL, OUT_FULL), np.float32)
    kl_val = 0.0
    for c in range(N_CORES):
        out[:, c * O_S : (c + 1) * O_S] = res.results[c]["out"]
        kl_val += float(res.results[c]["kl"][0, 0])
    if run_kwargs:
        kernel.last_results = res
    return out, np.float32(kl_val)
